# revision 21
# baseline (speedup 1.0000x reference)
"""Distributed (8-core) Trainium2 Bass kernel for speculative-decoding
rejection sampling (AscendRejectionSampler).

Strategy
--------
The algorithm only *needs* a tiny, data-dependent subset of the two huge
[512, 128000] probability tensors:
  * target/draft prob of each draft token  -> 512 scalar gathers each
  * argmax over ONE recovery row per request (<=128 rows of 128000)

So: shard the batch (128 requests -> 16 per core). Each core gets its own
rows of target/draft probs resident in DRAM, but only *reads* what it needs
via indirect DMA:
  1. gather tp/dp scalars at [token_row, draft_token_id] (one index per
     partition - HW indirect-DMA semantics), relayout to [16, 4] via a
     small SBUF->SBUF DMA
  2. run the accept/reject scan on-device (tensor_tensor_scan)
  3. compute the recovery row per request, replicate x8 via a PE matmul,
     and indirect-gather that row spread over 128 partitions
     (16 requests x 8 chunks of 16000, in 8 strips of 2000 for pipelining)
  4. per strip only a reduce_max (hidden under the strip DMAs); afterwards
     re-gather the winning strip per partition and run ONE max_index pass
     against the known max for the exact first-index-of-max (tie-safe)
  5. assemble the [16, 5] int32 output and DMA out.

HBM traffic per core: ~9MB instead of ~64MB (the headroom=8 win).
"""

import numpy as np
from contextlib import ExitStack

V = 128000            # vocab
B = 128               # requests
L = 4                 # max spec len
NCORES = 8
Q = B // NCORES       # 16 requests per core
NT = Q * L            # 64 tokens per core
OUTW = L + 1          # 5 output slots per request
CHUNKS = 8            # per-request recovery row split across partitions
WCH = V // CHUNKS     # 16000 elements per chunk
S = 8                 # strips per chunk (DMA/compute pipelining)
WS = WCH // S         # 2000 elements per strip
SUBW = 250            # sub-block width for the fine value pass
SENT_S = 65536.0      # sentinel > any strip base (s*WS <= 14000), f32-exact
SENT_C = 16777216.0   # sentinel > any vocab column (2^24), f32-exact

_GRAPH_CACHE = {}


def _build(R, debug=False):
    """Build + schedule + compile the per-core Bass graph for a shard with
    R resident prob rows (R=64 uniform path, R=80 ragged path)."""
    import concourse.bacc as bacc
    import concourse.tile as tile
    from concourse import bass, mybir

    dt = mybir.dt
    Alu = mybir.AluOpType
    Ax = mybir.AxisListType

    nc = bacc.Bacc("TRN2", target_bir_lowering=False, debug=False)

    def dbg(name, tile_ap, shape, dtype):
        if not debug:
            return
        d = nc.dram_tensor(f"dbg_{name}", shape, dtype,
                           kind="ExternalOutput").ap()
        nc.sync.dma_start(d[:], tile_ap)

    tgt = nc.dram_tensor("tgt", [2 * R, V], dt.float32,
                         kind="ExternalInput").ap()
    mi = nc.dram_tensor("meta_i", [2 * NT, 1], dt.int32,
                        kind="ExternalInput").ap()
    mf = nc.dram_tensor("meta_f", [Q, 24], dt.float32, kind="ExternalInput").ap()
    rmat = nc.dram_tensor("repmat", [Q, 128], dt.float32, kind="ExternalInput").ap()
    aux = nc.dram_tensor("aux128", [128, 65], dt.float32,
                         kind="ExternalInput").ap()
    idn = nc.dram_tensor("ident", [128, 128], dt.float32, kind="ExternalInput").ap()
    out = nc.dram_tensor("out", [Q, OUTW], dt.int32, kind="ExternalOutput").ap()

    with tile.TileContext(nc) as tc:
        with ExitStack() as ctx:
            sb = ctx.enter_context(tc.tile_pool(name="sb", bufs=1))
            pp = ctx.enter_context(tc.tile_pool(name="pp", bufs=1, space="PSUM"))
            stp = ctx.enter_context(tc.tile_pool(name="stp", bufs=6))

            # ---------------- metadata / constants ----------------
            # t_mi/t_mf ride the scalar HWDGE queue so the scalar-gather
            # and the scan don't wait on the bulkier constants below
            t_mi = sb.tile([2 * NT, 1], dt.int32)
            nc.scalar.dma_start(t_mi[:], mi[:])
            t_mf = sb.tile([Q, 24], dt.float32)
            nc.scalar.dma_start(t_mf[:], mf[:])
            t_rm = sb.tile([Q, 128], dt.float32)
            nc.sync.dma_start(t_rm[:], rmat[:])
            t_aux = sb.tile([128, 65], dt.float32)
            nc.sync.dma_start(t_aux[:], aux[:])
            t_id = sb.tile([128, 128], dt.float32)
            nc.sync.dma_start(t_id[:], idn[:])

            uef = t_mf[:, 0:4]          # uniform probs, invalid -> 1.0
            valid = t_mf[:, 4:8]        # (j < ndraft) as 1.0/0.0
            nd = t_mf[:, 8:9]
            bon = t_mf[:, 9:10]
            ploc = t_mf[:, 10:11]
            jp1 = t_mf[:, 11:15]        # j + 1
            col5 = t_mf[:, 15:20]       # 0..4
            d4f = t_mf[:, 20:24]        # draft token ids as f32

            t_co = t_aux[:, 0:1]        # (p % 8) * WCH
            sbMB = t_aux[:, 1:65]       # b*SUBW - SENT_S for the 64 sub-blocks

            # ---------------- token-level scalar gathers ----------------
            # one host-precomputed flat index per partition (t = 4q + j; tp
            # on partitions 0..63, dp on 64..127), then relayout to [16,4]
            # via tiny SBUF->SBUF DMAs on two HWDGE queues
            g128 = sb.tile([2 * NT, 1], dt.float32)
            nc.gpsimd.indirect_dma_start(
                out=g128[:], out_offset=None, in_=tgt,
                in_offset=bass.IndirectOffsetOnAxis(ap=t_mi[:], axis=1))
            tp = sb.tile([Q, 4], dt.float32)
            nc.sync.dma_start(tp[:], g128[0:NT, :])
            dp = sb.tile([Q, 4], dt.float32)
            nc.scalar.dma_start(dp[:], g128[NT:2 * NT, :])

            ones4 = sb.tile([Q, 4], dt.float32)
            nc.vector.memset(ones4[:], 1.0)
            im8 = sb.tile([128, 8], dt.float32)
            nc.vector.memset(im8[:], -3.0e38)
            dbg("tp", tp[:], [Q, 4], dt.float32)
            dbg("dp", dp[:], [Q, 4], dt.float32)

            # ---------------- rejection scan ----------------
            dpos = sb.tile([Q, 4], dt.float32)
            nc.vector.scalar_tensor_tensor(out=dpos[:], in0=dp[:], scalar=0.0,
                                           in1=valid, op0=Alu.is_gt,
                                           op1=Alu.mult)
            dpos_m = sb.tile([Q, 4], dt.uint8)
            nc.vector.tensor_copy(dpos_m[:], dpos[:])
            dps = sb.tile([Q, 4], dt.float32)
            nc.vector.select(dps[:], dpos_m[:], dp[:], ones4[:])
            # ratio = tp * (1/dps); InstReciprocal is the accurate HW
            # iterative divide
            rcp = sb.tile([Q, 4], dt.float32)
            nc.vector.reciprocal(rcp[:], dps[:])
            ratraw = sb.tile([Q, 4], dt.float32)
            nc.vector.tensor_tensor(out=ratraw[:], in0=tp[:], in1=rcp[:], op=Alu.mult)
            rat = sb.tile([Q, 4], dt.float32)
            nc.vector.select(rat[:], dpos_m[:], ratraw[:], ones4[:])

            # pi_t = min(pi_{t-1} * r_t, 1) ; U_t = U_{t-1} * u_t
            pi4 = sb.tile([Q, 4], dt.float32)
            nc.vector.tensor_tensor_scan(out=pi4[:], data0=rat[:], data1=ones4[:],
                                         initial=1.0, op0=Alu.mult, op1=Alu.min)
            U4 = sb.tile([Q, 4], dt.float32)
            nc.vector.tensor_tensor_scan(out=U4[:], data0=uef, data1=ones4[:],
                                         initial=1.0, op0=Alu.mult, op1=Alu.bypass)
            dposj = sb.tile([Q, 4], dt.float32)
            nc.vector.tensor_tensor(out=dposj[:], in0=dpos[:], in1=jp1, op=Alu.mult)
            ge4 = sb.tile([Q, 4], dt.float32)
            nc.vector.tensor_tensor(out=ge4[:], in0=pi4[:], in1=U4[:], op=Alu.is_ge)
            wacc = sb.tile([Q, 4], dt.float32)
            nc.vector.tensor_tensor(out=wacc[:], in0=ge4[:], in1=dposj[:],
                                    op=Alu.mult)
            lp1 = sb.tile([Q, 1], dt.float32)   # last + 1  (0 if none accepted)
            nc.vector.tensor_reduce(out=lp1[:], in_=wacc[:], axis=Ax.X, op=Alu.max)
            dbg("lp1", lp1[:], [Q, 1], dt.float32)

            # rejected = (nd > 0) & (last + 1 != nd)
            eql = sb.tile([Q, 1], dt.float32)
            nc.vector.tensor_tensor(out=eql[:], in0=lp1[:], in1=nd, op=Alu.is_equal)
            gt0 = sb.tile([Q, 1], dt.float32)
            nc.vector.tensor_scalar(out=gt0[:], in0=nd, scalar1=0.0,
                                    scalar2=None, op0=Alu.is_gt)
            rej = sb.tile([Q, 1], dt.float32)
            nc.vector.tensor_scalar(out=rej[:], in0=eql[:], scalar1=-1.0,
                                    scalar2=1.0, op0=Alu.mult, op1=Alu.add)
            nc.vector.tensor_tensor(out=rej[:], in0=rej[:], in1=gt0[:], op=Alu.mult)
            rej_m = sb.tile([Q, 1], dt.uint8)
            nc.vector.tensor_copy(rej_m[:], rej[:])
            wcol = sb.tile([Q, 1], dt.float32)
            nc.vector.select(wcol[:], rej_m[:], lp1[:], nd)
            dbg("rej", rej[:], [Q, 1], dt.float32)

            # recovery row (shard-local, clamped)
            rrow = sb.tile([Q, 1], dt.float32)
            nc.vector.tensor_tensor(out=rrow[:], in0=ploc, in1=lp1[:], op=Alu.add)
            nc.vector.tensor_scalar(out=rrow[:], in0=rrow[:], scalar1=float(R - 1),
                                    scalar2=0.0, op0=Alu.min, op1=Alu.max)
            dbg("rrow", rrow[:], [Q, 1], dt.float32)

            # replicate to 128 partitions: rep[p] = rrow[p // 8]
            prep = pp.tile([128, 1], dt.float32)
            nc.tensor.matmul(out=prep[:], lhsT=t_rm[:], rhs=rrow[:],
                             start=True, stop=True)
            bigf = sb.tile([128, 1], dt.float32)
            nc.vector.scalar_tensor_tensor(out=bigf[:], in0=prep[:],
                                           scalar=float(V), in1=t_co,
                                           op0=Alu.mult, op1=Alu.add)
            bigi = sb.tile([128, 1], dt.int32)
            nc.vector.tensor_copy(bigi[:], bigf[:])

            # ---------------- output prep (independent of the argmax) -------
            dz = sb.tile([Q, 5], dt.float32)
            nc.vector.tensor_copy(dz[:, 0:4], d4f)
            nc.vector.memset(dz[:, 4:5], 0.0)
            m1 = sb.tile([Q, 5], dt.float32)
            nc.vector.memset(m1[:], -1.0)
            keep = sb.tile([Q, 5], dt.uint8)
            nc.vector.tensor_tensor(out=keep[:], in0=col5,
                                    in1=lp1[:].to_broadcast([Q, 5]), op=Alu.is_lt)
            a5 = sb.tile([Q, 5], dt.float32)
            nc.vector.select(a5[:], keep[:], dz[:], m1[:])
            e5 = sb.tile([Q, 5], dt.uint8)
            nc.vector.tensor_tensor(out=e5[:], in0=col5,
                                    in1=wcol[:].to_broadcast([Q, 5]),
                                    op=Alu.is_equal)

            # ---------------- recovery-row gather: value pass ----------------
            # one reduce per strip, but over a [128, SUBS, SUBW] view so we
            # get per-sub-block maxima - the exact-index re-gather then only
            # needs SUBW elements per partition.
            NSUB = WCH // SUBW                   # 64 sub-blocks per chunk
            SPS = WS // SUBW                     # sub-blocks per strip
            SVf = sb.tile([128, NSUB], dt.float32)
            HWS = WS // 2
            HSPS = SPS // 2
            for h in range(2):   # strip 0 split in half: shorter ramp
                st = stp.tile([128, HWS], dt.float32, tag="hstrip")
                nc.gpsimd.indirect_dma_start(
                    out=st[:], out_offset=None, in_=tgt,
                    in_offset=bass.IndirectOffsetOnAxis(ap=bigi[:], axis=1),
                    element_offset=h * HWS)
                nc.vector.tensor_reduce(
                    out=SVf[:, h * HSPS:(h + 1) * HSPS],
                    in_=st[:].rearrange("p (a b) -> p a b", b=SUBW),
                    axis=Ax.X, op=Alu.max)
            for s in range(1, S):
                st = stp.tile([128, WS], dt.float32, tag="strip")
                nc.gpsimd.indirect_dma_start(
                    out=st[:], out_offset=None, in_=tgt,
                    in_offset=bass.IndirectOffsetOnAxis(ap=bigi[:], axis=1),
                    element_offset=s * WS)
                nc.vector.tensor_reduce(
                    out=SVf[:, s * SPS:(s + 1) * SPS],
                    in_=st[:].rearrange("p (a b) -> p a b", b=SUBW),
                    axis=Ax.X, op=Alu.max)
            dbg("SV", SVf[:], [128, NSUB], dt.float32)

            # chunk max + first sub-block achieving it
            mval = sb.tile([128, 1], dt.float32)
            nc.vector.tensor_reduce(out=mval[:], in_=SVf[:], axis=Ax.X, op=Alu.max)
            # value transpose + per-request max can start now (overlaps the
            # re-gather latency below)
            ptv = pp.tile([1, 128], dt.float32)
            nc.tensor.transpose(out=ptv[:], in_=mval[:], identity=t_id[:])
            ttv = sb.tile([1, 128], dt.float32)
            nc.vector.tensor_copy(ttv[:], ptv[:])
            r0v = ttv[0:1, :].rearrange("p (a b) -> p a b", b=CHUNKS)
            Mrq = sb.tile([1, Q], dt.float32)
            nc.vector.tensor_reduce(out=Mrq[:], in_=r0v, axis=Ax.X, op=Alu.max)
            nc.vector.tensor_copy(im8[:, 0:1], mval[:])

            cand = sb.tile([128, NSUB], dt.float32)
            nc.vector.scalar_tensor_tensor(out=cand[:], in0=SVf[:],
                                           scalar=mval[:], in1=sbMB,
                                           op0=Alu.is_equal, op1=Alu.mult)
            nc.vector.tensor_scalar_add(cand[:], cand[:], SENT_S)
            sWS = sb.tile([128, 1], dt.float32)  # (first sub-block) * SUBW
            nc.vector.tensor_reduce(out=sWS[:], in_=cand[:], axis=Ax.X,
                                    op=Alu.min)

            # re-gather the winning sub-block, find first index of max in it
            bigf2 = sb.tile([128, 1], dt.float32)
            nc.vector.tensor_tensor(out=bigf2[:], in0=bigf[:], in1=sWS[:],
                                    op=Alu.add)
            bigi2 = sb.tile([128, 1], dt.int32)
            nc.vector.tensor_copy(bigi2[:], bigf2[:])
            rst = sb.tile([128, SUBW], dt.float32)
            nc.gpsimd.indirect_dma_start(
                out=rst[:], out_offset=None, in_=tgt,
                in_offset=bass.IndirectOffsetOnAxis(ap=bigi2[:], axis=1))
            i8 = sb.tile([128, 8], dt.uint32)
            nc.vector.max_index(out=i8[:], in_max=im8[:], in_values=rst[:])
            i8f = sb.tile([128, 1], dt.float32)
            nc.vector.tensor_copy(i8f[:], i8[:, 0:1])
            gidx = sb.tile([128, 1], dt.float32)   # global column in the row
            nc.vector.scalar_tensor_tensor(out=gidx[:], in0=i8f[:],
                                           scalar=sWS[:], in1=t_co,
                                           op0=Alu.add, op1=Alu.add)

            pti = pp.tile([1, 128], dt.float32)
            nc.tensor.transpose(out=pti[:], in_=gidx[:], identity=t_id[:])
            tti = sb.tile([1, 128], dt.float32)
            nc.vector.tensor_copy(tti[:], pti[:])
            eqc = sb.tile([1, 128], dt.float32)
            eqcv = eqc[0:1, :].rearrange("p (a b) -> p a b", b=CHUNKS)
            nc.vector.tensor_tensor(out=eqcv, in0=r0v,
                                    in1=Mrq[:].to_broadcast([1, Q, CHUNKS]),
                                    op=Alu.is_equal)
            # candc = eqc * (col - 2^24) + 2^24  (exact for integer columns)
            candc = sb.tile([1, 128], dt.float32)
            nc.vector.scalar_tensor_tensor(out=candc[:], in0=tti[:],
                                           scalar=-SENT_C, in1=eqc[:],
                                           op0=Alu.add, op1=Alu.mult)
            nc.vector.tensor_scalar_add(candc[:], candc[:], SENT_C)
            rci = sb.tile([1, Q], dt.float32)
            nc.vector.tensor_reduce(out=rci[:],
                                    in_=candc[0:1, :].rearrange(
                                        "p (a b) -> p a b", b=CHUNKS),
                                    axis=Ax.X, op=Alu.min)
            dbg("rci", rci[:], [1, Q], dt.float32)

            # [1, Q] -> [Q, 1] (transpose mode is exact data movement)
            prc = pp.tile([Q, 1], dt.float32)
            nc.tensor.transpose(out=prc[:], in_=rci[:], identity=t_id[0:1, 0:1])
            rec16 = sb.tile([Q, 1], dt.float32)
            nc.vector.tensor_copy(rec16[:], prc[:])

            # ---------------- output assembly ----------------
            wval = sb.tile([Q, 1], dt.float32)
            nc.vector.select(wval[:], rej_m[:], rec16[:], bon)
            outf = sb.tile([Q, 5], dt.float32)
            nc.vector.select(outf[:], e5[:], wval[:].to_broadcast([Q, 5]), a5[:])
            outi = sb.tile([Q, 5], dt.int32)
            nc.vector.tensor_copy(outi[:], outf[:])
            nc.sync.dma_start(out[:], outi[:])

    nc.compile()
    return nc


def _get_graph(R, debug=False):
    key = (R, debug)
    if key not in _GRAPH_CACHE:
        _GRAPH_CACHE[key] = _build(R, debug=debug)
    return _GRAPH_CACHE[key]


def _prepare(draft_probs, target_probs, uniform_probs, draft_token_ids,
             cu_num_draft_tokens, bonus_token_ids):
    """Shard the full inputs into 8 per-core input maps. Returns (in_maps, R)."""
    target_probs = np.asarray(target_probs, dtype=np.float32)
    draft_probs = np.asarray(draft_probs, dtype=np.float32)
    uniform_probs = np.asarray(uniform_probs, dtype=np.float32)
    d_ids = np.asarray(draft_token_ids, dtype=np.int32)
    cu = np.asarray(cu_num_draft_tokens, dtype=np.int64)
    bonus = np.asarray(bonus_token_ids, dtype=np.int32)

    nt = target_probs.shape[0]
    assert cu.shape[0] == B
    prev = np.concatenate([np.zeros(1, np.int64), cu[:-1]])
    nd = cu - prev
    uniform = (nt == B * L) and bool(np.all(nd == L))

    if uniform:
        R = nt // NCORES            # 64 rows/core, zero-copy slices
        stride = L
    else:
        R = Q * (L + 1)             # 80 canonical rows/core (host row-gather)
        stride = L + 1

    # constants shared by all cores
    repmat = np.zeros((Q, 128), np.float32)
    for k in range(Q):
        repmat[k, k * CHUNKS:(k + 1) * CHUNKS] = 1.0
    aux128 = np.zeros((128, 65), np.float32)
    aux128[:, 0] = (np.arange(128) % CHUNKS) * WCH
    for b in range(WCH // SUBW):
        aux128[:, 1 + b] = b * SUBW - SENT_S
    ident = np.eye(128, dtype=np.float32)
    jp1 = np.tile(np.arange(1, L + 1, dtype=np.float32), (Q, 1))
    col5 = np.tile(np.arange(L + 1, dtype=np.float32), (Q, 1))
    tokrow_loc = (np.arange(Q)[:, None] * stride + np.arange(L)[None, :])

    in_maps = []
    for c in range(NCORES):
        qs = slice(c * Q, (c + 1) * Q)
        prev_c = prev[qs]
        nd_c = nd[qs]
        if uniform:
            row0 = c * Q * L
            tgt_c = target_probs[row0:row0 + R]
            drf_c = draft_probs[row0:row0 + R]
            d4 = d_ids[row0:row0 + Q * L].reshape(Q, L)
            uu4 = uniform_probs[row0:row0 + Q * L].reshape(Q, L)
            ploc = (np.arange(Q) * L).astype(np.float32)
        else:
            rows = np.clip(prev_c[:, None] + np.arange(L + 1)[None, :], 0, nt - 1)
            rows_flat = rows.reshape(-1)
            tgt_c = np.ascontiguousarray(target_probs[rows_flat])
            drf_c = np.ascontiguousarray(draft_probs[rows_flat])
            tokidx = np.clip(prev_c[:, None] + np.arange(L)[None, :], 0, nt - 1)
            d4 = d_ids[tokidx]
            uu4 = uniform_probs[tokidx]
            ploc = (np.arange(Q) * (L + 1)).astype(np.float32)

        validm = (np.arange(L)[None, :] < nd_c[:, None])
        comb = np.concatenate([tgt_c, drf_c], axis=0)
        meta_i = np.zeros((2 * NT, 1), np.int32)
        rowsV = (tokrow_loc * V).astype(np.int64).reshape(-1)
        meta_i[0:NT, 0] = (rowsV + d4.reshape(-1)).astype(np.int32)
        meta_i[NT:, 0] = (rowsV + R * V + d4.reshape(-1)).astype(np.int32)
        meta_f = np.zeros((Q, 24), np.float32)
        meta_f[:, 0:4] = np.where(validm, uu4, np.float32(1.0))
        meta_f[:, 4:8] = validm.astype(np.float32)
        meta_f[:, 8] = nd_c.astype(np.float32)
        meta_f[:, 9] = bonus[qs].astype(np.float32)
        meta_f[:, 10] = ploc
        meta_f[:, 11:15] = jp1
        meta_f[:, 15:20] = col5
        meta_f[:, 20:24] = d4.astype(np.float32)
        in_maps.append({
            "tgt": comb, "meta_i": meta_i, "meta_f": meta_f,
            "repmat": repmat, "aux128": aux128, "ident": ident,
        })
    return in_maps, R


def _run(in_maps, R, trace=False):
    from concourse.bass_utils import run_bass_kernel_spmd
    nc = _get_graph(R)
    res = run_bass_kernel_spmd(nc, in_maps, core_ids=list(range(NCORES)),
                               trace=trace)
    outs = [np.asarray(res.results[i]["out"]) for i in range(NCORES)]
    full = np.concatenate(outs, axis=0).astype(np.int32)
    return full, res


def kernel(draft_probs, target_probs, uniform_probs, draft_token_ids,
           cu_num_draft_tokens, bonus_token_ids):
    in_maps, R = _prepare(draft_probs, target_probs, uniform_probs,
                          draft_token_ids, cu_num_draft_tokens, bonus_token_ids)
    full, _ = _run(in_maps, R, trace=False)
    return full


def kernel_profiled(**inputs):
    """Like kernel() but with NTFF tracing; returns (out, exec_time_ns)."""
    in_maps, R = _prepare(**inputs)
    full, res = _run(in_maps, R, trace=True)
    return full, res.exec_time_ns


# revision 22
# speedup vs baseline: 1.0381x; 1.0381x over previous
"""Distributed (8-core) Trainium2 Bass kernel for speculative-decoding
rejection sampling (AscendRejectionSampler).

Strategy
--------
The algorithm only *needs* a tiny, data-dependent subset of the two huge
[512, 128000] probability tensors:
  * target/draft prob of each draft token  -> 512 scalar gathers each
  * argmax over ONE recovery row per request (<=128 rows of 128000)

So: shard the batch (128 requests -> 16 per core). Each core gets its own
rows of target/draft probs resident in DRAM, but only *reads* what it needs
via indirect DMA:
  1. gather tp/dp scalars at [token_row, draft_token_id] (one index per
     partition - HW indirect-DMA semantics), relayout to [16, 4] via a
     small SBUF->SBUF DMA
  2. run the accept/reject scan on-device (tensor_tensor_scan)
  3. compute the recovery row per request, replicate x8 via a PE matmul,
     and indirect-gather that row spread over 128 partitions
     (16 requests x 8 chunks of 16000, in 8 strips of 2000 for pipelining)
  4. per strip only a reduce_max (hidden under the strip DMAs); afterwards
     re-gather the winning strip per partition and run ONE max_index pass
     against the known max for the exact first-index-of-max (tie-safe)
  5. assemble the [16, 5] int32 output and DMA out.

HBM traffic per core: ~9MB instead of ~64MB (the headroom=8 win).
"""

import numpy as np
from contextlib import ExitStack

V = 128000            # vocab
B = 128               # requests
L = 4                 # max spec len
NCORES = 8
Q = B // NCORES       # 16 requests per core
NT = Q * L            # 64 tokens per core
OUTW = L + 1          # 5 output slots per request
CHUNKS = 8            # per-request recovery row split across partitions
WCH = V // CHUNKS     # 16000 elements per chunk
S = 8                 # strips per chunk (DMA/compute pipelining)
WS = WCH // S         # 2000 elements per strip
SUBW = 250            # sub-block width for the fine value pass
SENT_S = 65536.0      # sentinel > any strip base (s*WS <= 14000), f32-exact
SENT_C = 16777216.0   # sentinel > any vocab column (2^24), f32-exact

_GRAPH_CACHE = {}


def _build(R, debug=False):
    """Build + schedule + compile the per-core Bass graph for a shard with
    R resident prob rows (R=64 uniform path, R=80 ragged path)."""
    import concourse.bacc as bacc
    import concourse.tile as tile
    from concourse import bass, mybir

    dt = mybir.dt
    Alu = mybir.AluOpType
    Ax = mybir.AxisListType

    nc = bacc.Bacc("TRN2", target_bir_lowering=False, debug=False)

    def dbg(name, tile_ap, shape, dtype):
        if not debug:
            return
        d = nc.dram_tensor(f"dbg_{name}", shape, dtype,
                           kind="ExternalOutput").ap()
        nc.sync.dma_start(d[:], tile_ap)

    tgt = nc.dram_tensor("tgt", [2 * R, V], dt.float32,
                         kind="ExternalInput").ap()
    mi = nc.dram_tensor("meta_i", [2 * NT, 1], dt.int32,
                        kind="ExternalInput").ap()
    mf = nc.dram_tensor("meta_f", [Q, 24], dt.float32, kind="ExternalInput").ap()
    rmat = nc.dram_tensor("repmat", [Q, 128], dt.float32, kind="ExternalInput").ap()
    aux = nc.dram_tensor("aux128", [128, 65], dt.float32,
                         kind="ExternalInput").ap()
    idn = nc.dram_tensor("ident", [128, 128], dt.float32, kind="ExternalInput").ap()
    out = nc.dram_tensor("out", [Q, OUTW], dt.int32, kind="ExternalOutput").ap()

    with tile.TileContext(nc) as tc:
        with ExitStack() as ctx:
            sb = ctx.enter_context(tc.tile_pool(name="sb", bufs=1))
            pp = ctx.enter_context(tc.tile_pool(name="pp", bufs=1, space="PSUM"))
            stp = ctx.enter_context(tc.tile_pool(name="stp", bufs=6))

            # ---------------- metadata / constants ----------------
            # t_mi/t_mf ride the scalar HWDGE queue so the scalar-gather
            # and the scan don't wait on the bulkier constants below
            t_mi = sb.tile([2 * NT, 1], dt.int32)
            nc.scalar.dma_start(t_mi[:], mi[:])
            t_mf = sb.tile([Q, 24], dt.float32)
            nc.scalar.dma_start(t_mf[:], mf[:])
            t_rm = sb.tile([Q, 128], dt.float32)
            nc.sync.dma_start(t_rm[:], rmat[:])
            t_aux = sb.tile([128, 65], dt.float32)
            nc.sync.dma_start(t_aux[:], aux[:])
            t_id = sb.tile([128, 128], dt.float32)
            nc.sync.dma_start(t_id[:], idn[:])

            uef = t_mf[:, 0:4]          # uniform probs, invalid -> 1.0
            valid = t_mf[:, 4:8]        # (j < ndraft) as 1.0/0.0
            nd = t_mf[:, 8:9]
            bon = t_mf[:, 9:10]
            ploc = t_mf[:, 10:11]
            jp1 = t_mf[:, 11:15]        # j + 1
            col5 = t_mf[:, 15:20]       # 0..4
            d4f = t_mf[:, 20:24]        # draft token ids as f32

            t_co = t_aux[:, 0:1]        # (p % 8) * WCH
            sbMB = t_aux[:, 1:65]       # b*SUBW - SENT_S for the 64 sub-blocks

            # ---------------- token-level scalar gathers ----------------
            # one host-precomputed flat index per partition (t = 4q + j; tp
            # on partitions 0..63, dp on 64..127), then relayout to [16,4]
            # via tiny SBUF->SBUF DMAs on two HWDGE queues
            g128 = sb.tile([2 * NT, 1], dt.float32)
            nc.gpsimd.indirect_dma_start(
                out=g128[:], out_offset=None, in_=tgt,
                in_offset=bass.IndirectOffsetOnAxis(ap=t_mi[:], axis=1))
            tp = sb.tile([Q, 4], dt.float32)
            nc.sync.dma_start(tp[:], g128[0:NT, :])
            dp = sb.tile([Q, 4], dt.float32)
            nc.scalar.dma_start(dp[:], g128[NT:2 * NT, :])

            ones4 = sb.tile([Q, 4], dt.float32)
            nc.vector.memset(ones4[:], 1.0)
            im8 = sb.tile([128, 8], dt.float32)
            nc.vector.memset(im8[:], -3.0e38)
            dbg("tp", tp[:], [Q, 4], dt.float32)
            dbg("dp", dp[:], [Q, 4], dt.float32)

            # ---------------- rejection scan ----------------
            dpos = sb.tile([Q, 4], dt.float32)
            nc.vector.scalar_tensor_tensor(out=dpos[:], in0=dp[:], scalar=0.0,
                                           in1=valid, op0=Alu.is_gt,
                                           op1=Alu.mult)
            dpos_m = sb.tile([Q, 4], dt.uint8)
            nc.vector.tensor_copy(dpos_m[:], dpos[:])
            dps = sb.tile([Q, 4], dt.float32)
            nc.vector.select(dps[:], dpos_m[:], dp[:], ones4[:])
            # ratio = tp * (1/dps); InstReciprocal is the accurate HW
            # iterative divide
            rcp = sb.tile([Q, 4], dt.float32)
            nc.vector.reciprocal(rcp[:], dps[:])
            ratraw = sb.tile([Q, 4], dt.float32)
            nc.vector.tensor_tensor(out=ratraw[:], in0=tp[:], in1=rcp[:], op=Alu.mult)
            rat = sb.tile([Q, 4], dt.float32)
            nc.vector.select(rat[:], dpos_m[:], ratraw[:], ones4[:])

            # pi_t = min(pi_{t-1} * r_t, 1) ; U_t = U_{t-1} * u_t
            pi4 = sb.tile([Q, 4], dt.float32)
            nc.vector.tensor_tensor_scan(out=pi4[:], data0=rat[:], data1=ones4[:],
                                         initial=1.0, op0=Alu.mult, op1=Alu.min)
            U4 = sb.tile([Q, 4], dt.float32)
            nc.vector.tensor_tensor_scan(out=U4[:], data0=uef, data1=ones4[:],
                                         initial=1.0, op0=Alu.mult, op1=Alu.bypass)
            dposj = sb.tile([Q, 4], dt.float32)
            nc.vector.tensor_tensor(out=dposj[:], in0=dpos[:], in1=jp1, op=Alu.mult)
            ge4 = sb.tile([Q, 4], dt.float32)
            nc.vector.tensor_tensor(out=ge4[:], in0=pi4[:], in1=U4[:], op=Alu.is_ge)
            wacc = sb.tile([Q, 4], dt.float32)
            nc.vector.tensor_tensor(out=wacc[:], in0=ge4[:], in1=dposj[:],
                                    op=Alu.mult)
            lp1 = sb.tile([Q, 1], dt.float32)   # last + 1  (0 if none accepted)
            nc.vector.tensor_reduce(out=lp1[:], in_=wacc[:], axis=Ax.X, op=Alu.max)
            dbg("lp1", lp1[:], [Q, 1], dt.float32)

            # rejected = (nd > 0) & (last + 1 != nd)
            eql = sb.tile([Q, 1], dt.float32)
            nc.vector.tensor_tensor(out=eql[:], in0=lp1[:], in1=nd, op=Alu.is_equal)
            gt0 = sb.tile([Q, 1], dt.float32)
            nc.vector.tensor_scalar(out=gt0[:], in0=nd, scalar1=0.0,
                                    scalar2=None, op0=Alu.is_gt)
            rej = sb.tile([Q, 1], dt.float32)
            nc.vector.tensor_scalar(out=rej[:], in0=eql[:], scalar1=-1.0,
                                    scalar2=1.0, op0=Alu.mult, op1=Alu.add)
            nc.vector.tensor_tensor(out=rej[:], in0=rej[:], in1=gt0[:], op=Alu.mult)
            rej_m = sb.tile([Q, 1], dt.uint8)
            nc.vector.tensor_copy(rej_m[:], rej[:])
            wcol = sb.tile([Q, 1], dt.float32)
            nc.vector.select(wcol[:], rej_m[:], lp1[:], nd)
            dbg("rej", rej[:], [Q, 1], dt.float32)

            # recovery row (shard-local, clamped)
            rrow = sb.tile([Q, 1], dt.float32)
            nc.vector.tensor_tensor(out=rrow[:], in0=ploc, in1=lp1[:], op=Alu.add)
            nc.vector.tensor_scalar(out=rrow[:], in0=rrow[:], scalar1=float(R - 1),
                                    scalar2=0.0, op0=Alu.min, op1=Alu.max)
            dbg("rrow", rrow[:], [Q, 1], dt.float32)

            # replicate to 128 partitions: rep[p] = rrow[p // 8]
            prep = pp.tile([128, 1], dt.float32)
            nc.tensor.matmul(out=prep[:], lhsT=t_rm[:], rhs=rrow[:],
                             start=True, stop=True)
            bigf = sb.tile([128, 1], dt.float32)
            nc.vector.scalar_tensor_tensor(out=bigf[:], in0=prep[:],
                                           scalar=float(V), in1=t_co,
                                           op0=Alu.mult, op1=Alu.add)
            bigi = sb.tile([128, 1], dt.int32)
            nc.vector.tensor_copy(bigi[:], bigf[:])

            # ---------------- output prep (independent of the argmax) -------
            dz = sb.tile([Q, 5], dt.float32)
            nc.vector.tensor_copy(dz[:, 0:4], d4f)
            nc.vector.memset(dz[:, 4:5], 0.0)
            m1 = sb.tile([Q, 5], dt.float32)
            nc.vector.memset(m1[:], -1.0)
            keep = sb.tile([Q, 5], dt.uint8)
            nc.vector.tensor_tensor(out=keep[:], in0=col5,
                                    in1=lp1[:].to_broadcast([Q, 5]), op=Alu.is_lt)
            a5 = sb.tile([Q, 5], dt.float32)
            nc.vector.select(a5[:], keep[:], dz[:], m1[:])
            e5 = sb.tile([Q, 5], dt.uint8)
            nc.vector.tensor_tensor(out=e5[:], in0=col5,
                                    in1=wcol[:].to_broadcast([Q, 5]),
                                    op=Alu.is_equal)

            # ---------------- recovery-row gather: value pass ----------------
            # one reduce per strip, but over a [128, SUBS, SUBW] view so we
            # get per-sub-block maxima - the exact-index re-gather then only
            # needs SUBW elements per partition.
            NSUB = WCH // SUBW                   # 64 sub-blocks per chunk
            SPS = WS // SUBW                     # sub-blocks per strip
            SVf = sb.tile([128, NSUB], dt.float32)
            NS2 = 2 * S                  # 16 half-width strips
            WS2 = WS // 2                # 1000 elements each
            SPS2 = WS2 // SUBW
            for s in range(NS2):
                st = stp.tile([128, WS2], dt.float32, tag="strip")
                nc.gpsimd.indirect_dma_start(
                    out=st[:], out_offset=None, in_=tgt,
                    in_offset=bass.IndirectOffsetOnAxis(ap=bigi[:], axis=1),
                    element_offset=s * WS2)
                nc.vector.tensor_reduce(
                    out=SVf[:, s * SPS2:(s + 1) * SPS2],
                    in_=st[:].rearrange("p (a b) -> p a b", b=SUBW),
                    axis=Ax.X, op=Alu.max)
            dbg("SV", SVf[:], [128, NSUB], dt.float32)

            # chunk max + first sub-block achieving it
            mval = sb.tile([128, 1], dt.float32)
            nc.vector.tensor_reduce(out=mval[:], in_=SVf[:], axis=Ax.X, op=Alu.max)
            # value transpose + per-request max can start now (overlaps the
            # re-gather latency below)
            ptv = pp.tile([1, 128], dt.float32)
            nc.tensor.transpose(out=ptv[:], in_=mval[:], identity=t_id[:])
            ttv = sb.tile([1, 128], dt.float32)
            nc.vector.tensor_copy(ttv[:], ptv[:])
            r0v = ttv[0:1, :].rearrange("p (a b) -> p a b", b=CHUNKS)
            Mrq = sb.tile([1, Q], dt.float32)
            nc.vector.tensor_reduce(out=Mrq[:], in_=r0v, axis=Ax.X, op=Alu.max)
            nc.vector.tensor_copy(im8[:, 0:1], mval[:])

            cand = sb.tile([128, NSUB], dt.float32)
            nc.vector.scalar_tensor_tensor(out=cand[:], in0=SVf[:],
                                           scalar=mval[:], in1=sbMB,
                                           op0=Alu.is_equal, op1=Alu.mult)
            nc.vector.tensor_scalar_add(cand[:], cand[:], SENT_S)
            sWS = sb.tile([128, 1], dt.float32)  # (first sub-block) * SUBW
            nc.vector.tensor_reduce(out=sWS[:], in_=cand[:], axis=Ax.X,
                                    op=Alu.min)

            # re-gather the winning sub-block, find first index of max in it
            bigf2 = sb.tile([128, 1], dt.float32)
            nc.vector.tensor_tensor(out=bigf2[:], in0=bigf[:], in1=sWS[:],
                                    op=Alu.add)
            bigi2 = sb.tile([128, 1], dt.int32)
            nc.vector.tensor_copy(bigi2[:], bigf2[:])
            rst = sb.tile([128, SUBW], dt.float32)
            nc.gpsimd.indirect_dma_start(
                out=rst[:], out_offset=None, in_=tgt,
                in_offset=bass.IndirectOffsetOnAxis(ap=bigi2[:], axis=1))
            i8 = sb.tile([128, 8], dt.uint32)
            nc.vector.max_index(out=i8[:], in_max=im8[:], in_values=rst[:])
            i8f = sb.tile([128, 1], dt.float32)
            nc.vector.tensor_copy(i8f[:], i8[:, 0:1])
            gidx = sb.tile([128, 1], dt.float32)   # global column in the row
            nc.vector.scalar_tensor_tensor(out=gidx[:], in0=i8f[:],
                                           scalar=sWS[:], in1=t_co,
                                           op0=Alu.add, op1=Alu.add)

            pti = pp.tile([1, 128], dt.float32)
            nc.tensor.transpose(out=pti[:], in_=gidx[:], identity=t_id[:])
            tti = sb.tile([1, 128], dt.float32)
            nc.vector.tensor_copy(tti[:], pti[:])
            eqc = sb.tile([1, 128], dt.float32)
            eqcv = eqc[0:1, :].rearrange("p (a b) -> p a b", b=CHUNKS)
            nc.vector.tensor_tensor(out=eqcv, in0=r0v,
                                    in1=Mrq[:].to_broadcast([1, Q, CHUNKS]),
                                    op=Alu.is_equal)
            # candc = eqc * (col - 2^24) + 2^24  (exact for integer columns)
            candc = sb.tile([1, 128], dt.float32)
            nc.vector.scalar_tensor_tensor(out=candc[:], in0=tti[:],
                                           scalar=-SENT_C, in1=eqc[:],
                                           op0=Alu.add, op1=Alu.mult)
            nc.vector.tensor_scalar_add(candc[:], candc[:], SENT_C)
            rci = sb.tile([1, Q], dt.float32)
            nc.vector.tensor_reduce(out=rci[:],
                                    in_=candc[0:1, :].rearrange(
                                        "p (a b) -> p a b", b=CHUNKS),
                                    axis=Ax.X, op=Alu.min)
            dbg("rci", rci[:], [1, Q], dt.float32)

            # [1, Q] -> [Q, 1] (transpose mode is exact data movement)
            prc = pp.tile([Q, 1], dt.float32)
            nc.tensor.transpose(out=prc[:], in_=rci[:], identity=t_id[0:1, 0:1])
            rec16 = sb.tile([Q, 1], dt.float32)
            nc.vector.tensor_copy(rec16[:], prc[:])

            # ---------------- output assembly ----------------
            wval = sb.tile([Q, 1], dt.float32)
            nc.vector.select(wval[:], rej_m[:], rec16[:], bon)
            outf = sb.tile([Q, 5], dt.float32)
            nc.vector.select(outf[:], e5[:], wval[:].to_broadcast([Q, 5]), a5[:])
            outi = sb.tile([Q, 5], dt.int32)
            nc.vector.tensor_copy(outi[:], outf[:])
            nc.sync.dma_start(out[:], outi[:])

    nc.compile()
    return nc


def _get_graph(R, debug=False):
    key = (R, debug)
    if key not in _GRAPH_CACHE:
        _GRAPH_CACHE[key] = _build(R, debug=debug)
    return _GRAPH_CACHE[key]


def _prepare(draft_probs, target_probs, uniform_probs, draft_token_ids,
             cu_num_draft_tokens, bonus_token_ids):
    """Shard the full inputs into 8 per-core input maps. Returns (in_maps, R)."""
    target_probs = np.asarray(target_probs, dtype=np.float32)
    draft_probs = np.asarray(draft_probs, dtype=np.float32)
    uniform_probs = np.asarray(uniform_probs, dtype=np.float32)
    d_ids = np.asarray(draft_token_ids, dtype=np.int32)
    cu = np.asarray(cu_num_draft_tokens, dtype=np.int64)
    bonus = np.asarray(bonus_token_ids, dtype=np.int32)

    nt = target_probs.shape[0]
    assert cu.shape[0] == B
    prev = np.concatenate([np.zeros(1, np.int64), cu[:-1]])
    nd = cu - prev
    uniform = (nt == B * L) and bool(np.all(nd == L))

    if uniform:
        R = nt // NCORES            # 64 rows/core, zero-copy slices
        stride = L
    else:
        R = Q * (L + 1)             # 80 canonical rows/core (host row-gather)
        stride = L + 1

    # constants shared by all cores
    repmat = np.zeros((Q, 128), np.float32)
    for k in range(Q):
        repmat[k, k * CHUNKS:(k + 1) * CHUNKS] = 1.0
    aux128 = np.zeros((128, 65), np.float32)
    aux128[:, 0] = (np.arange(128) % CHUNKS) * WCH
    for b in range(WCH // SUBW):
        aux128[:, 1 + b] = b * SUBW - SENT_S
    ident = np.eye(128, dtype=np.float32)
    jp1 = np.tile(np.arange(1, L + 1, dtype=np.float32), (Q, 1))
    col5 = np.tile(np.arange(L + 1, dtype=np.float32), (Q, 1))
    tokrow_loc = (np.arange(Q)[:, None] * stride + np.arange(L)[None, :])

    in_maps = []
    for c in range(NCORES):
        qs = slice(c * Q, (c + 1) * Q)
        prev_c = prev[qs]
        nd_c = nd[qs]
        if uniform:
            row0 = c * Q * L
            tgt_c = target_probs[row0:row0 + R]
            drf_c = draft_probs[row0:row0 + R]
            d4 = d_ids[row0:row0 + Q * L].reshape(Q, L)
            uu4 = uniform_probs[row0:row0 + Q * L].reshape(Q, L)
            ploc = (np.arange(Q) * L).astype(np.float32)
        else:
            rows = np.clip(prev_c[:, None] + np.arange(L + 1)[None, :], 0, nt - 1)
            rows_flat = rows.reshape(-1)
            tgt_c = np.ascontiguousarray(target_probs[rows_flat])
            drf_c = np.ascontiguousarray(draft_probs[rows_flat])
            tokidx = np.clip(prev_c[:, None] + np.arange(L)[None, :], 0, nt - 1)
            d4 = d_ids[tokidx]
            uu4 = uniform_probs[tokidx]
            ploc = (np.arange(Q) * (L + 1)).astype(np.float32)

        validm = (np.arange(L)[None, :] < nd_c[:, None])
        comb = np.concatenate([tgt_c, drf_c], axis=0)
        meta_i = np.zeros((2 * NT, 1), np.int32)
        rowsV = (tokrow_loc * V).astype(np.int64).reshape(-1)
        meta_i[0:NT, 0] = (rowsV + d4.reshape(-1)).astype(np.int32)
        meta_i[NT:, 0] = (rowsV + R * V + d4.reshape(-1)).astype(np.int32)
        meta_f = np.zeros((Q, 24), np.float32)
        meta_f[:, 0:4] = np.where(validm, uu4, np.float32(1.0))
        meta_f[:, 4:8] = validm.astype(np.float32)
        meta_f[:, 8] = nd_c.astype(np.float32)
        meta_f[:, 9] = bonus[qs].astype(np.float32)
        meta_f[:, 10] = ploc
        meta_f[:, 11:15] = jp1
        meta_f[:, 15:20] = col5
        meta_f[:, 20:24] = d4.astype(np.float32)
        in_maps.append({
            "tgt": comb, "meta_i": meta_i, "meta_f": meta_f,
            "repmat": repmat, "aux128": aux128, "ident": ident,
        })
    return in_maps, R


def _run(in_maps, R, trace=False):
    from concourse.bass_utils import run_bass_kernel_spmd
    nc = _get_graph(R)
    res = run_bass_kernel_spmd(nc, in_maps, core_ids=list(range(NCORES)),
                               trace=trace)
    outs = [np.asarray(res.results[i]["out"]) for i in range(NCORES)]
    full = np.concatenate(outs, axis=0).astype(np.int32)
    return full, res


def kernel(draft_probs, target_probs, uniform_probs, draft_token_ids,
           cu_num_draft_tokens, bonus_token_ids):
    in_maps, R = _prepare(draft_probs, target_probs, uniform_probs,
                          draft_token_ids, cu_num_draft_tokens, bonus_token_ids)
    full, _ = _run(in_maps, R, trace=False)
    return full


def kernel_profiled(**inputs):
    """Like kernel() but with NTFF tracing; returns (out, exec_time_ns)."""
    in_maps, R = _prepare(**inputs)
    full, res = _run(in_maps, R, trace=True)
    return full, res.exec_time_ns
